# revision 4
# baseline (speedup 1.0000x reference)
"""Binarized dense layer for Trainium2 (8 NeuronCores).

Computes y = sign(x) @ sign(w) + b with sign(v) = -1 if v < 0 else +1,
matching jnp.where(v < 0, -1, 1) (including v == +0.0 -> +1).

Full shapes: x [8192, 2048] f32, w [2048, 2048] f32, b [2048] f32
-> y [8192, 2048] f32.

Sharding: rows of x across 8 cores (data parallel); w is K-sharded for
the *fetch* (core c reads rows [c*256, (c+1)*256) of w = 2 MB instead of
16 MB), binarized to fp8 locally, and exchanged via 4 pipelined
DRAM->DRAM AllGathers (one per 512-column quarter, fp8, Shared output)
so every core ends with the full binarized weight matrix while HBM
weight traffic drops ~2.5x per core.

Per-core pipeline:
  W: DMA f32 k-slice -> ScalarE Sign -> fp8 [128,2,512] per quarter ->
     bounce to DRAM -> AllGather (8 cores) -> wq [128,16,512] fp8 SBUF.
  X: DMA f32 row-chunks. k-blocks 0..xbar_kb-1 take the DMA-xbar route:
     ScalarE Sign -> bf16 -> dma_start_transpose -> DVE cast -> fp8.
     Remaining k-blocks go PE transpose (f32) -> ScalarE Sign-evac ->
     fp8. This splits transpose work between the DMA fabric and the PE
     so neither is the bottleneck.
  Matmul: fp8 DoubleRow (256-row contraction, exact +-1 products, f32
     PSUM). 8 accumulating matmuls per (m-tile, quarter) into one PSUM
     bank; 6 opsum + 2 tpsum banks.
  Epilogue: VectorE adds broadcast bias, GpSimd SWDGE writes y.
"""
import numpy as np

import concourse.bass as bass
import concourse.mybir as mybir
import concourse.tile as tile
from concourse import bacc
from concourse.masks import make_identity

F32 = mybir.dt.float32
BF16 = mybir.dt.bfloat16
FP8 = mybir.dt.float8e4
P = 128
NQT = 512
Sign = mybir.ActivationFunctionType.Sign


def _build_kernel(M=1024, K=2048, N=2048, n_cores=8, tp_w=True, xbar_kb=12,
                  xstage_bufs=4, wq_bufs=4, tpsum_bufs=2, opsum_bufs=6,
                  osb_bufs=4, out_eng='gpsimd'):
    KS = K // P            # 16 k-subtiles
    KP = KS // 2           # 8 DoubleRow matmuls per group
    MT = M // P            # 8 m-tiles
    NQ = N // NQT          # 4 column quarters
    KC = K // n_cores      # 256 k-rows fetched per core
    AQ = KC // P           # 2 k-subtiles fetched per core
    PEB = KS - xbar_kb     # k-blocks per m-tile via PE transpose

    nc = bacc.Bacc("TRN2", target_bir_lowering=False, debug=False,
                   num_devices=n_cores)
    x = nc.dram_tensor("x", [M, K], F32, kind="ExternalInput").ap()
    if tp_w:
        w = nc.dram_tensor("w", [KC, N], F32, kind="ExternalInput").ap()
    else:
        w = nc.dram_tensor("w", [K, N], F32, kind="ExternalInput").ap()
    b = nc.dram_tensor("b", [N], F32, kind="ExternalInput").ap()
    y = nc.dram_tensor("y", [M, N], F32, kind="ExternalOutput").ap()
    w_r = w.rearrange("(a p) n -> p a n", p=P)

    with tile.TileContext(nc) as tc:
        with (
            tc.tile_pool(name="cst", bufs=1) as cst,
            tc.tile_pool(name="wstage", bufs=1) as wstage,
            tc.tile_pool(name="wsb", bufs=1) as wsbp,
            tc.tile_pool(name="wq", bufs=wq_bufs) as wqp,
            tc.tile_pool(name="dram", bufs=1, space="DRAM") as dram,
            tc.tile_pool(name="xstage", bufs=xstage_bufs) as xstage,
            tc.tile_pool(name="xsb", bufs=3) as xsbp,
            tc.tile_pool(name="xtb", bufs=3) as xtbp,
            tc.tile_pool(name="xbt", bufs=1) as xbtp,
            tc.tile_pool(name="osb", bufs=osb_bufs) as osbp,
            tc.tile_pool(name="tpsum", bufs=tpsum_bufs, space="PSUM") as tpsum,
            tc.tile_pool(name="opsum", bufs=opsum_bufs, space="PSUM") as opsum,
        ):
            eps = cst.tile([P, 1], F32, tag="eps")
            nc.vector.memset(eps[:], 1e-30)
            ident = None
            if PEB > 0:
                ident = cst.tile([P, P], F32, tag="ident")
                make_identity(nc, ident[:])
            bias_q = [cst.tile([P, NQT], F32, tag=f"bias{q}", name=f"bias{q}")
                      for q in range(NQ)]

            # ---------------- W path ----------------
            wq = []
            if tp_w:
                ws = wstage.tile([P, AQ, N], F32, tag="ws", name="ws")
                wsb = wsbp.tile([P, AQ, N], FP8, tag="wsb", name="wsb")
                for q in range(NQ):
                    # fetch this core's k-slice of quarter q first
                    nc.sync.dma_start(ws[:, :, q * NQT:(q + 1) * NQT],
                                      w_r[:, :, q * NQT:(q + 1) * NQT])
                for q in range(NQ):
                    qs = slice(q * NQT, (q + 1) * NQT)
                    nc.scalar.activation(wsb[:, :, qs], ws[:, :, qs],
                                         Sign, bias=eps[:])
                    bounce = dram.tile([P, AQ, NQT], FP8, tag=f"bnc{q}",
                                       name=f"bnc{q}")
                    nc.gpsimd.dma_start(bounce[:], wsb[:, :, qs])
                    gathered = dram.tile([n_cores, P, AQ, NQT], FP8,
                                         tag=f"gat{q}", name=f"gat{q}",
                                         addr_space="Shared")
                    nc.gpsimd.collective_compute(
                        "AllGather", mybir.AluOpType.bypass,
                        replica_groups=[list(range(n_cores))],
                        ins=[bounce[:]], outs=[gathered[:]])
                    wt = wqp.tile([P, KS, NQT], FP8, tag="wq", name=f"wq{q}")
                    nc.gpsimd.dma_start(
                        wt[:].rearrange("p (c a) n -> p c a n", c=n_cores),
                        gathered[:].rearrange("c p a n -> p c a n"))
                    wq.append(wt)
            else:
                for q in range(NQ):
                    ws = wstage.tile([P, KS, NQT], F32, tag="ws",
                                     name=f"ws{q}", bufs=2)
                    nc.sync.dma_start(
                        ws[:], w_r[:, :, q * NQT:(q + 1) * NQT])
                    wt = wqp.tile([P, KS, NQT], FP8, tag="wq", name=f"wq{q}")
                    nc.scalar.activation(wt[:], ws[:], Sign, bias=eps[:])
                    wq.append(wt)

            for q in range(NQ):
                nc.sync.dma_start(
                    bias_q[q][:],
                    b[None, q * NQT:(q + 1) * NQT].to_broadcast([P, NQT]))

            # ---------------- X path ----------------
            xbt = [xbtp.tile([P, KS, P], FP8, tag=f"xbt{mi}",
                             name=f"xbt{mi}") for mi in range(MT)]

            def load_x(mi):
                xs = xstage.tile([P, K], F32, tag="xs", name=f"xs{mi}")
                nc.sync.dma_start(xs[:], x[mi * P:(mi + 1) * P, :])
                return xs

            def prep_x_xbar(mi, xs):
                """k-blocks [0, xbar_kb): sign->bf16, xbar transpose, cast."""
                if xbar_kb == 0:
                    return
                kw = xbar_kb * P
                xsb = xsbp.tile([P, kw], BF16, tag="xsb", name=f"xsb{mi}")
                nc.scalar.activation(xsb[:], xs[:, :kw], Sign, bias=eps[:])
                xtb = xtbp.tile([P, xbar_kb, P], BF16, tag="xtb",
                                name=f"xtb{mi}")
                for kb in range(xbar_kb):
                    nc.scalar.dma_start_transpose(
                        xtb[:, kb, :], xsb[:, kb * P:(kb + 1) * P])
                nc.vector.tensor_copy(
                    xbt[mi][:, 0:xbar_kb, :],
                    xtb[:].rearrange("p k m -> p (k m)")
                          .rearrange("p (k m) -> p k m", k=xbar_kb))

            def prep_x_pe(mi, xs):
                """k-blocks [xbar_kb, KS): PE transpose f32 + Sign evac."""
                if PEB == 0:
                    return
                pt = tpsum.tile([P, PEB * P], F32, tag="tp", name=f"tp{mi}")
                for j in range(PEB):
                    kj = xbar_kb + j
                    nc.tensor.transpose(pt[:, j * P:(j + 1) * P],
                                        xs[:, kj * P:(kj + 1) * P], ident[:])
                nc.scalar.activation(
                    xbt[mi][:, xbar_kb:KS, :],
                    pt[:].rearrange("p (a m) -> p a m", a=PEB),
                    Sign, bias=eps[:])

            xss = []
            for mi in range(MT):
                xs = load_x(mi)
                xss.append(xs)
                prep_x_xbar(mi, xs)

            # ---------------- Matmul schedule ----------------
            # Interleave the PE transposes of later m-tiles between early
            # matmul groups so the PE never waits on the x stream.
            def mm_group(q, mi):
                op = opsum.tile([P, NQT], F32, tag="op", name=f"op{mi}_{q}")
                for t in range(KP):
                    nc.tensor.matmul(
                        op[:],
                        lhsT=xbt[mi][:, 2 * t:2 * t + 2, :],
                        rhs=wq[q][:, 2 * t:2 * t + 2, :],
                        start=(t == 0), stop=(t == KP - 1),
                        perf_mode=mybir.MatmulPerfMode.DoubleRow)
                ob = osbp.tile([P, NQT], F32, tag="ob", name=f"ob{mi}_{q}")
                nc.vector.tensor_add(ob[:], op[:], bias_q[q][:])
                getattr(nc, out_eng).dma_start(
                    y[mi * P:(mi + 1) * P, q * NQT:(q + 1) * NQT], ob[:])

            if PEB > 0:
                for mi in range(4):
                    prep_x_pe(mi, xss[mi])
                for mi in range(MT):
                    if mi + 4 < MT:
                        prep_x_pe(mi + 4, xss[mi + 4])
                    mm_group(0, mi)
            else:
                for mi in range(MT):
                    mm_group(0, mi)
            for q in range(1, NQ):
                for mi in range(MT):
                    mm_group(q, mi)
    nc.compile()
    return nc


N_CORES = 8
M_FULL, K_DIM, N_DIM = 8192, 2048, 2048
M_LOC = M_FULL // N_CORES
K_LOC = K_DIM // N_CORES
TP_W = True
_nc_cache = {}


def _get_nc():
    if "nc" not in _nc_cache:
        _nc_cache["nc"] = _build_kernel(M=M_LOC, K=K_DIM, N=N_DIM,
                                        n_cores=N_CORES, tp_w=TP_W)
    return _nc_cache["nc"]


def _in_maps(inputs, kernel, bias):
    maps = []
    for c in range(N_CORES):
        wmap = (kernel[c * K_LOC:(c + 1) * K_LOC, :] if TP_W else kernel)
        maps.append({"x": inputs[c * M_LOC:(c + 1) * M_LOC, :],
                     "w": wmap, "b": bias})
    return maps


def kernel(inputs: np.ndarray, kernel: np.ndarray, bias: np.ndarray) -> np.ndarray:
    assert inputs.shape == (M_FULL, K_DIM) and inputs.dtype == np.float32
    assert kernel.shape == (K_DIM, N_DIM) and kernel.dtype == np.float32
    assert bias.shape == (N_DIM,) and bias.dtype == np.float32
    nc = _get_nc()
    in_maps = _in_maps(inputs, kernel, bias)
    try:
        from concourse.bass_utils import run_bass_kernel_spmd
        results = run_bass_kernel_spmd(
            nc, in_maps, core_ids=list(range(N_CORES))).results
    except Exception:
        from concourse import bass2jax
        bass2jax.install_neuronx_cc_hook()
        results = bass2jax.run_bass_via_pjrt(nc, in_maps, n_cores=N_CORES)
    return np.concatenate([r["y"] for r in results], axis=0)


# revision 8
# speedup vs baseline: 1.7719x; 1.7719x over previous
"""Binarized dense layer for Trainium2 (8 NeuronCores).

Computes y = sign(x) @ sign(w) + b with sign(v) = -1 if v < 0 else +1,
matching jnp.where(v < 0, -1, 1) (including v == +0.0 -> +1).

Full shapes: x [8192, 2048] f32, w [2048, 2048] f32, b [2048] f32
-> y [8192, 2048] f32.

Sharding: rows of x across 8 cores (data parallel); w is K-sharded for
the *fetch* (core c reads rows [c*256, (c+1)*256) of w = 2 MB instead of
16 MB), binarized to fp8 locally, and exchanged via 4 pipelined
DRAM->DRAM AllGathers (one per 512-column quarter, fp8, Shared output)
so every core ends with the full binarized weight matrix while HBM
weight traffic drops ~2.5x per core.

Per-core pipeline:
  W: DMA f32 k-slice -> ScalarE Sign -> fp8 [128,2,512] per quarter ->
     bounce to DRAM -> AllGather (8 cores) -> wq [128,16,512] fp8 SBUF.
  X: DMA f32 row-chunks. k-blocks 0..xbar_kb-1 take the DMA-xbar route:
     ScalarE Sign -> bf16 -> dma_start_transpose -> DVE cast -> fp8.
     Remaining k-blocks go PE transpose (f32) -> ScalarE Sign-evac ->
     fp8. This splits transpose work between the DMA fabric and the PE
     so neither is the bottleneck.
  Matmul: fp8 DoubleRow (256-row contraction, exact +-1 products, f32
     PSUM). 8 accumulating matmuls per (m-tile, quarter) into one PSUM
     bank; 6 opsum + 2 tpsum banks.
  Epilogue: VectorE adds broadcast bias, GpSimd SWDGE writes y.
"""
import numpy as np

import concourse.bass as bass
import concourse.mybir as mybir
import concourse.tile as tile
from concourse import bacc
from concourse.masks import make_identity

F32 = mybir.dt.float32
BF16 = mybir.dt.bfloat16
FP8 = mybir.dt.float8e4
P = 128
NQT = 512
Sign = mybir.ActivationFunctionType.Sign


def _build_kernel(M=1024, K=2048, N=2048, n_cores=8, tp_w=True, xbar_kb=0,
                  xstage_bufs=4, wq_bufs=4, tpsum_bufs=2, opsum_bufs=6,
                  osb_bufs=4, out_eng='gpsimd'):
    KS = K // P            # 16 k-subtiles
    KP = KS // 2           # 8 DoubleRow matmuls per group
    MT = M // P            # 8 m-tiles
    NQ = N // NQT          # 4 column quarters
    KC = K // n_cores      # 256 k-rows fetched per core
    AQ = KC // P           # 2 k-subtiles fetched per core
    PEB = KS - xbar_kb     # k-blocks per m-tile via PE transpose

    nc = bacc.Bacc("TRN2", target_bir_lowering=False, debug=False,
                   num_devices=n_cores)
    x = nc.dram_tensor("x", [M, K], F32, kind="ExternalInput").ap()
    if tp_w:
        w = nc.dram_tensor("w", [KC, N], F32, kind="ExternalInput").ap()
    else:
        w = nc.dram_tensor("w", [K, N], F32, kind="ExternalInput").ap()
    b = nc.dram_tensor("b", [N], F32, kind="ExternalInput").ap()
    y = nc.dram_tensor("y", [M, N], F32, kind="ExternalOutput").ap()
    w_r = w.rearrange("(a p) n -> p a n", p=P)

    with tile.TileContext(nc) as tc:
        with (
            tc.tile_pool(name="cst", bufs=1) as cst,
            tc.tile_pool(name="wstage", bufs=1) as wstage,
            tc.tile_pool(name="wsb", bufs=1) as wsbp,
            tc.tile_pool(name="wq", bufs=wq_bufs) as wqp,
            tc.tile_pool(name="dram", bufs=1, space="DRAM") as dram,
            tc.tile_pool(name="xstage", bufs=xstage_bufs) as xstage,
            tc.tile_pool(name="xsb", bufs=3) as xsbp,
            tc.tile_pool(name="xtb", bufs=3) as xtbp,
            tc.tile_pool(name="xbt", bufs=1) as xbtp,
            tc.tile_pool(name="osb", bufs=osb_bufs) as osbp,
            tc.tile_pool(name="tpsum", bufs=tpsum_bufs, space="PSUM") as tpsum,
            tc.tile_pool(name="opsum", bufs=opsum_bufs, space="PSUM") as opsum,
        ):
            eps = cst.tile([P, 1], F32, tag="eps")
            nc.vector.memset(eps[:], 1e-30)
            ident = None
            if PEB > 0:
                ident = cst.tile([P, P], F32, tag="ident")
                make_identity(nc, ident[:])
            bias_q = [cst.tile([P, NQT], F32, tag=f"bias{q}", name=f"bias{q}")
                      for q in range(NQ)]

            # ---------------- W path ----------------
            wq = []
            if tp_w:
                # Tiny warmup AllGather to absorb NRT CC channel setup
                # latency before the real weight exchanges.
                wrm_in = dram.tile([P, 16], FP8, tag="wrm_i", name="wrm_i")
                wrm_out = dram.tile([n_cores, P, 16], FP8, tag="wrm_o",
                                    name="wrm_o", addr_space="Shared")
                wrm_sb = cst.tile([P, 16], FP8, tag="wrm_sb")
                nc.vector.memset(wrm_sb[:], 1.0)
                nc.gpsimd.dma_start(wrm_in[:], wrm_sb[:])
                nc.gpsimd.collective_compute(
                    "AllGather", mybir.AluOpType.bypass,
                    replica_groups=[list(range(n_cores))],
                    ins=[wrm_in[:]], outs=[wrm_out[:]])
                ws = wstage.tile([P, AQ, N], F32, tag="ws", name="ws")
                wsb = wsbp.tile([P, AQ, N], FP8, tag="wsb", name="wsb")
                for q in range(NQ):
                    # fetch this core's k-slice of quarter q first
                    nc.sync.dma_start(ws[:, :, q * NQT:(q + 1) * NQT],
                                      w_r[:, :, q * NQT:(q + 1) * NQT])
                for q in range(NQ):
                    qs = slice(q * NQT, (q + 1) * NQT)
                    nc.scalar.activation(wsb[:, :, qs], ws[:, :, qs],
                                         Sign, bias=eps[:])
                    bounce = dram.tile([P, AQ, NQT], FP8, tag=f"bnc{q}",
                                       name=f"bnc{q}")
                    nc.gpsimd.dma_start(bounce[:], wsb[:, :, qs])
                    gathered = dram.tile([n_cores, P, AQ, NQT], FP8,
                                         tag=f"gat{q}", name=f"gat{q}",
                                         addr_space="Shared")
                    nc.gpsimd.collective_compute(
                        "AllGather", mybir.AluOpType.bypass,
                        replica_groups=[list(range(n_cores))],
                        ins=[bounce[:]], outs=[gathered[:]])
                    wt = wqp.tile([P, KS, NQT], FP8, tag="wq", name=f"wq{q}")
                    nc.gpsimd.dma_start(
                        wt[:].rearrange("p (c a) n -> p c a n", c=n_cores),
                        gathered[:].rearrange("c p a n -> p c a n"))
                    wq.append(wt)
            else:
                for q in range(NQ):
                    ws = wstage.tile([P, KS, NQT], F32, tag="ws",
                                     name=f"ws{q}", bufs=2)
                    nc.sync.dma_start(
                        ws[:], w_r[:, :, q * NQT:(q + 1) * NQT])
                    wt = wqp.tile([P, KS, NQT], FP8, tag="wq", name=f"wq{q}")
                    nc.scalar.activation(wt[:], ws[:], Sign, bias=eps[:])
                    wq.append(wt)

            for q in range(NQ):
                nc.sync.dma_start(
                    bias_q[q][:],
                    b[None, q * NQT:(q + 1) * NQT].to_broadcast([P, NQT]))

            # ---------------- X path ----------------
            xbt = [xbtp.tile([P, KS, P], FP8, tag=f"xbt{mi}",
                             name=f"xbt{mi}") for mi in range(MT)]

            def load_x(mi):
                xs = xstage.tile([P, K], F32, tag="xs", name=f"xs{mi}")
                nc.sync.dma_start(xs[:], x[mi * P:(mi + 1) * P, :])
                return xs

            def prep_x_xbar(mi, xs):
                """k-blocks [0, xbar_kb): sign->bf16, xbar transpose, cast."""
                if xbar_kb == 0:
                    return
                kw = xbar_kb * P
                xsb = xsbp.tile([P, kw], BF16, tag="xsb", name=f"xsb{mi}")
                nc.scalar.activation(xsb[:], xs[:, :kw], Sign, bias=eps[:])
                xtb = xtbp.tile([P, xbar_kb, P], BF16, tag="xtb",
                                name=f"xtb{mi}")
                for kb in range(xbar_kb):
                    nc.scalar.dma_start_transpose(
                        xtb[:, kb, :], xsb[:, kb * P:(kb + 1) * P])
                nc.vector.tensor_copy(
                    xbt[mi][:, 0:xbar_kb, :],
                    xtb[:].rearrange("p k m -> p (k m)")
                          .rearrange("p (k m) -> p k m", k=xbar_kb))

            def prep_x_pe(mi, xs):
                """k-blocks [xbar_kb, KS): PE transpose f32 + Sign evac,
                in groups of <=4 blocks (one PSUM bank each)."""
                for g0 in range(xbar_kb, KS, 4):
                    gn = min(4, KS - g0)
                    pt = tpsum.tile([P, gn * P], F32, tag="tp",
                                    name=f"tp{mi}_{g0}")
                    for j in range(gn):
                        kj = g0 + j
                        nc.tensor.transpose(pt[:, j * P:(j + 1) * P],
                                            xs[:, kj * P:(kj + 1) * P],
                                            ident[:])
                    nc.scalar.activation(
                        xbt[mi][:, g0:g0 + gn, :],
                        pt[:].rearrange("p (a m) -> p a m", a=gn),
                        Sign, bias=eps[:])

            xss = []
            for mi in range(MT):
                xs = load_x(mi)
                xss.append(xs)
                prep_x_xbar(mi, xs)

            # ---------------- Matmul schedule ----------------
            # Interleave the PE transposes of later m-tiles between early
            # matmul groups so the PE never waits on the x stream.
            def mm_group(q, mi):
                op = opsum.tile([P, NQT], F32, tag="op", name=f"op{mi}_{q}")
                for t in range(KP):
                    nc.tensor.matmul(
                        op[:],
                        lhsT=xbt[mi][:, 2 * t:2 * t + 2, :],
                        rhs=wq[q][:, 2 * t:2 * t + 2, :],
                        start=(t == 0), stop=(t == KP - 1),
                        perf_mode=mybir.MatmulPerfMode.DoubleRow)
                ob = osbp.tile([P, NQT], F32, tag="ob", name=f"ob{mi}_{q}")
                nc.vector.tensor_add(ob[:], op[:], bias_q[q][:])
                getattr(nc, out_eng).dma_start(
                    y[mi * P:(mi + 1) * P, q * NQT:(q + 1) * NQT], ob[:])

            if PEB > 0:
                for mi in range(5):
                    prep_x_pe(mi, xss[mi])
                for mi in range(MT):
                    if mi + 5 < MT:
                        prep_x_pe(mi + 5, xss[mi + 5])
                    mm_group(0, mi)
            else:
                for mi in range(MT):
                    mm_group(0, mi)
            for q in range(1, NQ):
                for mi in range(MT):
                    mm_group(q, mi)
    nc.compile()
    return nc


N_CORES = 8
M_FULL, K_DIM, N_DIM = 8192, 2048, 2048
M_LOC = M_FULL // N_CORES
K_LOC = K_DIM // N_CORES
TP_W = True
_nc_cache = {}


def _get_nc():
    if "nc" not in _nc_cache:
        _nc_cache["nc"] = _build_kernel(M=M_LOC, K=K_DIM, N=N_DIM,
                                        n_cores=N_CORES, tp_w=TP_W)
    return _nc_cache["nc"]


def _in_maps(inputs, kernel, bias):
    maps = []
    for c in range(N_CORES):
        wmap = (kernel[c * K_LOC:(c + 1) * K_LOC, :] if TP_W else kernel)
        maps.append({"x": inputs[c * M_LOC:(c + 1) * M_LOC, :],
                     "w": wmap, "b": bias})
    return maps


def kernel(inputs: np.ndarray, kernel: np.ndarray, bias: np.ndarray) -> np.ndarray:
    assert inputs.shape == (M_FULL, K_DIM) and inputs.dtype == np.float32
    assert kernel.shape == (K_DIM, N_DIM) and kernel.dtype == np.float32
    assert bias.shape == (N_DIM,) and bias.dtype == np.float32
    nc = _get_nc()
    in_maps = _in_maps(inputs, kernel, bias)
    try:
        from concourse.bass_utils import run_bass_kernel_spmd
        results = run_bass_kernel_spmd(
            nc, in_maps, core_ids=list(range(N_CORES))).results
    except Exception:
        from concourse import bass2jax
        bass2jax.install_neuronx_cc_hook()
        results = bass2jax.run_bass_via_pjrt(nc, in_maps, n_cores=N_CORES)
    return np.concatenate([r["y"] for r in results], axis=0)


# revision 10
# speedup vs baseline: 1.8518x; 1.0450x over previous
"""Binarized dense layer for Trainium2 (8 NeuronCores).

Computes y = sign(x) @ sign(w) + b with sign(v) = -1 if v < 0 else +1,
matching jnp.where(v < 0, -1, 1) (including v == +0.0 -> +1).

Full shapes: x [8192, 2048] f32, w [2048, 2048] f32, b [2048] f32
-> y [8192, 2048] f32.

Sharding: rows of x across 8 cores (data parallel). The weight fetch is
K-sharded (core c reads rows [c*256, (c+1)*256) of w = 2 MB instead of
16 MB), binarized to fp8 locally, and exchanged via 4 pipelined
DRAM->DRAM AllGathers (one per 512-column quarter, Shared output), so
every core ends with the full binarized weight matrix while per-core
HBM weight traffic drops ~2.5x. A tiny warmup AllGather issued at t~0
absorbs the ~20us NRT collective-channel setup so the real exchanges
complete while the x path is still streaming/transposing.

Queue discipline (each engine queue is FIFO, so ordering is part of the
design): sync/SP carries x tiles + the w slice + bias + the gathered
readbacks (emitted last); GpSimd carries ONLY bounce writes and
collectives; scalar/Act carries Sign activations, PSUM sign-evacuations
and the y output DMAs; vector does the bias adds.

Per-core kernel:
  X: DMA f32 row-chunks -> PE transpose (128x128 f32 blocks, 4 per PSUM
     bank) -> ScalarE Sign (+1e-30 bias so sign(0)=+1) evacuates to fp8
     [k-part, k-subtile, m] tiles.
  Matmul: fp8 DoubleRow (256-row contraction per instruction; +-1
     products exact, f32 PSUM accumulation, |sums| <= 2048 exact).
     8 accumulating matmuls per (m-tile, quarter) PSUM bank; PE
     transposes of later m-tiles are interleaved between early matmul
     groups so the PE never idles on the x stream.
  Epilogue: VectorE adds the partition-broadcast bias, ScalarE SWDGE
     writes y.
"""
import numpy as np

import concourse.bass as bass
import concourse.mybir as mybir
import concourse.tile as tile
from concourse import bacc
from concourse.masks import make_identity

F32 = mybir.dt.float32
FP8 = mybir.dt.float8e4
P = 128
NQT = 512
Sign = mybir.ActivationFunctionType.Sign


def _build_kernel(M=1024, K=2048, N=2048, n_cores=8, tp_w=True,
                  xstage_bufs=8, tpsum_bufs=2, opsum_bufs=6,
                  osb_bufs=4, out_eng='scalar', n_prelead=4):
    KS = K // P            # 16 k-subtiles
    KP = KS // 2           # 8 DoubleRow matmuls per group
    MT = M // P            # 8 m-tiles
    NQ = N // NQT          # 4 column quarters
    KC = K // n_cores      # 256 k-rows fetched per core
    AQ = KC // P           # 2 k-subtiles fetched per core

    nc = bacc.Bacc("TRN2", target_bir_lowering=False, debug=False,
                   num_devices=n_cores)
    x = nc.dram_tensor("x", [M, K], F32, kind="ExternalInput").ap()
    if tp_w:
        w = nc.dram_tensor("w", [KC, N], F32, kind="ExternalInput").ap()
    else:
        w = nc.dram_tensor("w", [K, N], F32, kind="ExternalInput").ap()
    b = nc.dram_tensor("b", [N], F32, kind="ExternalInput").ap()
    y = nc.dram_tensor("y", [M, N], F32, kind="ExternalOutput").ap()
    w_r = w.rearrange("(a p) n -> p a n", p=P)

    with tile.TileContext(nc) as tc:
        with (
            tc.tile_pool(name="cst", bufs=1) as cst,
            tc.tile_pool(name="wstage", bufs=1) as wstage,
            tc.tile_pool(name="wsb", bufs=1) as wsbp,
            tc.tile_pool(name="wq", bufs=4) as wqp,
            tc.tile_pool(name="dram", bufs=1, space="DRAM") as dram,
            tc.tile_pool(name="xstage", bufs=xstage_bufs) as xstage,
            tc.tile_pool(name="xbt", bufs=1) as xbtp,
            tc.tile_pool(name="osb", bufs=osb_bufs) as osbp,
            tc.tile_pool(name="tpsum", bufs=tpsum_bufs, space="PSUM") as tpsum,
            tc.tile_pool(name="opsum", bufs=opsum_bufs, space="PSUM") as opsum,
        ):
            eps = cst.tile([P, 1], F32, tag="eps")
            nc.vector.memset(eps[:], 1e-30)
            ident = cst.tile([P, P], F32, tag="ident")
            make_identity(nc, ident[:])
            bias_q = [cst.tile([P, NQT], F32, tag=f"bias{q}", name=f"bias{q}")
                      for q in range(NQ)]

            xbt = [xbtp.tile([P, KS, P], FP8, tag=f"xbt{mi}",
                             name=f"xbt{mi}") for mi in range(MT)]

            def load_x(mi):
                xs = xstage.tile([P, K], F32, tag="xs", name=f"xs{mi}")
                nc.sync.dma_start(xs[:], x[mi * P:(mi + 1) * P, :])
                return xs

            # ---------------- W path ----------------
            xss = [load_x(0)]     # first x tile ahead of the w slice
            wq = []
            read_gathered = None
            if tp_w:
                # warmup AllGather: absorbs CC channel setup latency
                wrm_in = dram.tile([P, 16], FP8, tag="wrm_i", name="wrm_i")
                wrm_out = dram.tile([n_cores, P, 16], FP8, tag="wrm_o",
                                    name="wrm_o", addr_space="Shared")
                wrm_sb = cst.tile([P, 16], FP8, tag="wrm_sb")
                nc.vector.memset(wrm_sb[:], 1.0)
                nc.gpsimd.dma_start(wrm_in[:], wrm_sb[:])
                nc.gpsimd.collective_compute(
                    "AllGather", mybir.AluOpType.bypass,
                    replica_groups=[list(range(n_cores))],
                    ins=[wrm_in[:]], outs=[wrm_out[:]])

                ws = wstage.tile([P, AQ, N], F32, tag="ws", name="ws")
                nc.sync.dma_start(ws[:], w_r[:])
                wsb = wsbp.tile([P, AQ, N], FP8, tag="wsb", name="wsb")
                gathered_q = []
                for q in range(NQ):
                    qs = slice(q * NQT, (q + 1) * NQT)
                    nc.scalar.activation(wsb[:, :, qs], ws[:, :, qs],
                                         Sign, bias=eps[:])
                    bounce = dram.tile([P, AQ, NQT], FP8, tag=f"bnc{q}",
                                       name=f"bnc{q}")
                    nc.gpsimd.dma_start(bounce[:], wsb[:, :, qs])
                    gathered = dram.tile([n_cores, P, AQ, NQT], FP8,
                                         tag=f"gat{q}", name=f"gat{q}",
                                         addr_space="Shared")
                    nc.gpsimd.collective_compute(
                        "AllGather", mybir.AluOpType.bypass,
                        replica_groups=[list(range(n_cores))],
                        ins=[bounce[:]], outs=[gathered[:]])
                    gathered_q.append(gathered)
                    wq.append(wqp.tile([P, KS, NQT], FP8, tag="wq",
                                       name=f"wq{q}"))

                def read_gathered(q):
                    nc.sync.dma_start(
                        wq[q][:].rearrange("p (c a) n -> p c a n",
                                           c=n_cores),
                        gathered_q[q][:].rearrange("c p a n -> p c a n"))
            else:
                for q in range(NQ):
                    ws = wstage.tile([P, KS, NQT], F32, tag="wsr",
                                     name=f"ws{q}", bufs=2)
                    nc.sync.dma_start(
                        ws[:], w_r[:, :, q * NQT:(q + 1) * NQT])
                    wt = wqp.tile([P, KS, NQT], FP8, tag="wq", name=f"wq{q}")
                    nc.scalar.activation(wt[:], ws[:], Sign, bias=eps[:])
                    wq.append(wt)

            # ---------------- X loads, bias, gathered readbacks ----------
            for mi in range(1, MT):
                xss.append(load_x(mi))
            for q in range(NQ):
                nc.sync.dma_start(
                    bias_q[q][:],
                    b[None, q * NQT:(q + 1) * NQT].to_broadcast([P, NQT]))
            if tp_w:
                for q in range(NQ):
                    read_gathered(q)

            # ---------------- X prep (PE transpose + Sign evac) ----------
            def prep_x(mi):
                for g0 in range(0, KS, 4):
                    pt = tpsum.tile([P, 4 * P], F32, tag="tp",
                                    name=f"tp{mi}_{g0}")
                    for j in range(4):
                        kj = g0 + j
                        nc.tensor.transpose(pt[:, j * P:(j + 1) * P],
                                            xss[mi][:, kj * P:(kj + 1) * P],
                                            ident[:])
                    nc.scalar.activation(
                        xbt[mi][:, g0:g0 + 4, :],
                        pt[:].rearrange("p (a m) -> p a m", a=4),
                        Sign, bias=eps[:])

            # ---------------- Matmul schedule ----------------
            def mm_group(q, mi):
                op = opsum.tile([P, NQT], F32, tag="op", name=f"op{mi}_{q}")
                for t in range(KP):
                    nc.tensor.matmul(
                        op[:],
                        lhsT=xbt[mi][:, 2 * t:2 * t + 2, :],
                        rhs=wq[q][:, 2 * t:2 * t + 2, :],
                        start=(t == 0), stop=(t == KP - 1),
                        perf_mode=mybir.MatmulPerfMode.DoubleRow)
                ob = osbp.tile([P, NQT], F32, tag="ob", name=f"ob{mi}_{q}")
                nc.vector.tensor_add(ob[:], op[:], bias_q[q][:])
                getattr(nc, out_eng).dma_start(
                    y[mi * P:(mi + 1) * P, q * NQT:(q + 1) * NQT], ob[:])

            for mi in range(n_prelead):
                prep_x(mi)
            for mi in range(MT):
                if mi + n_prelead < MT:
                    prep_x(mi + n_prelead)
                mm_group(0, mi)
            for q in range(1, NQ):
                for mi in range(MT):
                    mm_group(q, mi)
    nc.compile()
    return nc


N_CORES = 8
M_FULL, K_DIM, N_DIM = 8192, 2048, 2048
M_LOC = M_FULL // N_CORES
K_LOC = K_DIM // N_CORES
TP_W = True
_nc_cache = {}


def _get_nc():
    if "nc" not in _nc_cache:
        _nc_cache["nc"] = _build_kernel(M=M_LOC, K=K_DIM, N=N_DIM,
                                        n_cores=N_CORES, tp_w=TP_W)
    return _nc_cache["nc"]


def _in_maps(inputs, kernel, bias):
    maps = []
    for c in range(N_CORES):
        wmap = (kernel[c * K_LOC:(c + 1) * K_LOC, :] if TP_W else kernel)
        maps.append({"x": inputs[c * M_LOC:(c + 1) * M_LOC, :],
                     "w": wmap, "b": bias})
    return maps


def kernel(inputs: np.ndarray, kernel: np.ndarray, bias: np.ndarray) -> np.ndarray:
    assert inputs.shape == (M_FULL, K_DIM) and inputs.dtype == np.float32
    assert kernel.shape == (K_DIM, N_DIM) and kernel.dtype == np.float32
    assert bias.shape == (N_DIM,) and bias.dtype == np.float32
    nc = _get_nc()
    in_maps = _in_maps(inputs, kernel, bias)
    try:
        from concourse.bass_utils import run_bass_kernel_spmd
        results = run_bass_kernel_spmd(
            nc, in_maps, core_ids=list(range(N_CORES))).results
    except Exception:
        from concourse import bass2jax
        bass2jax.install_neuronx_cc_hook()
        results = bass2jax.run_bass_via_pjrt(nc, in_maps, n_cores=N_CORES)
    return np.concatenate([r["y"] for r in results], axis=0)


# revision 11
# speedup vs baseline: 2.7576x; 1.4892x over previous
"""Binarized dense layer for Trainium2 (8 NeuronCores, data-parallel).

Computes y = sign(x) @ sign(w) + b with sign(v) = -1 if v < 0 else +1,
matching jnp.where(v < 0, -1, 1) (including v == +0.0 -> +1).

Full shapes: x [8192, 2048] f32, w [2048, 2048] f32, b [2048] f32
-> y [8192, 2048] f32. Rows of x are sharded across 8 cores; w, b are
replicated.

Design notes (trace-driven):
  - The kernel is jointly input-DMA-bound (24 MB f32 in per core at
    ~410 GB/s sustained) and PE-bound (256 fp8 DoubleRow matmuls at
    ~216 ns pitch + 128 f32 transposes at ~109 ns ~= 69 us), so the
    schedule keeps both saturated from t~0: x tiles and w quarter-chunks
    interleave on the sync HWDGE queue in exactly PE consumption order.
  - PE FIFO order = emission order: transposes of m-tile i+2 are
    interleaved between the first quarter's matmul groups, so the PE
    streams densely (no HAM re-throttle) from first x arrival to the
    last matmul.
  - Sign activations (w chunks) and PSUM sign-evacuations (x) share the
    scalar queue, emitted in data-arrival order; y outputs ride GpSimd
    SWDGE so they never contend with input loads for queue slots.
  - An AllGather-based tensor-parallel weight fetch was measured and
    rejected: each NRT collective costs ~20 us and they serialize
    (~140 us for 4), dwarfing the 14 MB of HBM traffic it saves.

Matmul: fp8e4 DoubleRow (256-row contraction per instruction; +-1
products exact, f32 PSUM accumulation, |sums| <= 2048 exact). 8
accumulating matmuls per (m-tile, quarter) PSUM bank, 6 opsum banks +
2 transpose banks. VectorE adds the partition-broadcast f32 bias
(rounding matches the reference exactly).
"""
import numpy as np

import concourse.bass as bass
import concourse.mybir as mybir
import concourse.tile as tile
from concourse import bacc
from concourse.masks import make_identity

F32 = mybir.dt.float32
FP8 = mybir.dt.float8e4
P = 128
NQT = 512
Sign = mybir.ActivationFunctionType.Sign


def _build_kernel(M=1024, K=2048, N=2048, n_cores=8,
                  xstage_bufs=8, tpsum_bufs=2, opsum_bufs=6,
                  osb_bufs=4, out_eng='gpsimd'):
    KS = K // P            # 16 k-subtiles
    KP = KS // 2           # 8 DoubleRow matmuls per group
    MT = M // P            # 8 m-tiles
    NQ = N // NQT          # 4 column quarters
    WG = 4                 # k-subtiles per w stage chunk

    nc = bacc.Bacc("TRN2", target_bir_lowering=False, debug=False,
                   num_devices=n_cores)
    x = nc.dram_tensor("x", [M, K], F32, kind="ExternalInput").ap()
    w = nc.dram_tensor("w", [K, N], F32, kind="ExternalInput").ap()
    b = nc.dram_tensor("b", [N], F32, kind="ExternalInput").ap()
    y = nc.dram_tensor("y", [M, N], F32, kind="ExternalOutput").ap()
    w_r = w.rearrange("(a p) n -> p a n", p=P)

    with tile.TileContext(nc) as tc:
        with (
            tc.tile_pool(name="cst", bufs=1) as cst,
            tc.tile_pool(name="wstage", bufs=6) as wstage,
            tc.tile_pool(name="wq", bufs=2) as wqp,
            tc.tile_pool(name="xstage", bufs=xstage_bufs) as xstage,
            tc.tile_pool(name="xbt", bufs=1) as xbtp,
            tc.tile_pool(name="osb", bufs=osb_bufs) as osbp,
            tc.tile_pool(name="tpsum", bufs=tpsum_bufs, space="PSUM") as tpsum,
            tc.tile_pool(name="opsum", bufs=opsum_bufs, space="PSUM") as opsum,
        ):
            eps = cst.tile([P, 1], F32, tag="eps")
            nc.vector.memset(eps[:], 1e-30)
            ident = cst.tile([P, P], F32, tag="ident")
            make_identity(nc, ident[:])
            bias_q = [cst.tile([P, NQT], F32, tag=f"bias{q}", name=f"bias{q}")
                      for q in range(NQ)]

            xbt = [xbtp.tile([P, KS, P], FP8, tag=f"xbt{mi}",
                             name=f"xbt{mi}") for mi in range(MT)]
            wq = [wqp.tile([P, KS, NQT], FP8, tag=f"wq{q % 2}",
                           name=f"wq{q}") for q in range(NQ)]
            xss = [None] * MT

            def load_x(mi):
                xs = xstage.tile([P, K], F32, tag="xs", name=f"xs{mi}")
                nc.sync.dma_start(xs[:], x[mi * P:(mi + 1) * P, :])
                xss[mi] = xs

            def load_w_chunk(q, g):
                ws = wstage.tile([P, WG, NQT], F32, tag="ws",
                                 name=f"ws{q}_{g}")
                nc.sync.dma_start(
                    ws[:], w_r[:, g * WG:(g + 1) * WG,
                               q * NQT:(q + 1) * NQT])
                return ws

            def sign_w_chunk(q, g, ws):
                nc.scalar.activation(wq[q][:, g * WG:(g + 1) * WG, :],
                                     ws[:], Sign, bias=eps[:])

            # ---- input stream: interleaved in PE-consumption order ----
            load_x(0)
            load_x(1)
            wchunks = {}
            for g in range(4):
                wchunks[(0, g)] = load_w_chunk(0, g)
            load_x(2)
            load_x(3)
            for g in range(4):
                wchunks[(1, g)] = load_w_chunk(1, g)
            load_x(4)
            load_x(5)
            for g in range(4):
                wchunks[(2, g)] = load_w_chunk(2, g)
            load_x(6)
            load_x(7)
            for g in range(4):
                wchunks[(3, g)] = load_w_chunk(3, g)
            for q in range(NQ):
                nc.sync.dma_start(
                    bias_q[q][:],
                    b[None, q * NQT:(q + 1) * NQT].to_broadcast([P, NQT]))

            # ---- X prep: PE transpose (4 blocks/bank) + Sign evac ----
            def prep_x(mi):
                for g0 in range(0, KS, 4):
                    pt = tpsum.tile([P, 4 * P], F32, tag="tp",
                                    name=f"tp{mi}_{g0}")
                    for j in range(4):
                        kj = g0 + j
                        nc.tensor.transpose(pt[:, j * P:(j + 1) * P],
                                            xss[mi][:, kj * P:(kj + 1) * P],
                                            ident[:])
                    nc.scalar.activation(
                        xbt[mi][:, g0:g0 + 4, :],
                        pt[:].rearrange("p (a m) -> p a m", a=4),
                        Sign, bias=eps[:])

            def mm_group(q, mi):
                op = opsum.tile([P, NQT], F32, tag="op", name=f"op{mi}_{q}")
                for t in range(KP):
                    nc.tensor.matmul(
                        op[:],
                        lhsT=xbt[mi][:, 2 * t:2 * t + 2, :],
                        rhs=wq[q][:, 2 * t:2 * t + 2, :],
                        start=(t == 0), stop=(t == KP - 1),
                        perf_mode=mybir.MatmulPerfMode.DoubleRow)
                ob = osbp.tile([P, NQT], F32, tag="ob", name=f"ob{mi}_{q}")
                nc.vector.tensor_add(ob[:], op[:], bias_q[q][:])
                getattr(nc, out_eng).dma_start(
                    y[mi * P:(mi + 1) * P, q * NQT:(q + 1) * NQT], ob[:])

            # scalar queue: w-quarter signs interleaved with x evacs in
            # arrival order; PE queue: preps interleaved with q0 groups.
            for g in range(4):
                sign_w_chunk(0, g, wchunks[(0, g)])
            prep_x(0)
            prep_x(1)
            for mi in range(MT):
                if mi == 0:
                    for g in range(4):
                        sign_w_chunk(1, g, wchunks[(1, g)])
                if mi == 2:
                    for g in range(4):
                        sign_w_chunk(2, g, wchunks[(2, g)])
                if mi == 4:
                    for g in range(4):
                        sign_w_chunk(3, g, wchunks[(3, g)])
                if mi + 2 < MT:
                    prep_x(mi + 2)
                mm_group(0, mi)
            for q in range(1, NQ):
                for mi in range(MT):
                    mm_group(q, mi)
    nc.compile()
    return nc


N_CORES = 8
M_FULL, K_DIM, N_DIM = 8192, 2048, 2048
M_LOC = M_FULL // N_CORES
K_LOC = K_DIM // N_CORES
TP_W = False
_nc_cache = {}


def _get_nc():
    if "nc" not in _nc_cache:
        _nc_cache["nc"] = _build_kernel(M=M_LOC, K=K_DIM, N=N_DIM,
                                        n_cores=N_CORES)
    return _nc_cache["nc"]


def _in_maps(inputs, kernel, bias):
    return [{"x": inputs[c * M_LOC:(c + 1) * M_LOC, :],
             "w": kernel, "b": bias} for c in range(N_CORES)]


def kernel(inputs: np.ndarray, kernel: np.ndarray, bias: np.ndarray) -> np.ndarray:
    assert inputs.shape == (M_FULL, K_DIM) and inputs.dtype == np.float32
    assert kernel.shape == (K_DIM, N_DIM) and kernel.dtype == np.float32
    assert bias.shape == (N_DIM,) and bias.dtype == np.float32
    nc = _get_nc()
    in_maps = _in_maps(inputs, kernel, bias)
    try:
        from concourse.bass_utils import run_bass_kernel_spmd
        results = run_bass_kernel_spmd(
            nc, in_maps, core_ids=list(range(N_CORES))).results
    except Exception:
        from concourse import bass2jax
        bass2jax.install_neuronx_cc_hook()
        results = bass2jax.run_bass_via_pjrt(nc, in_maps, n_cores=N_CORES)
    return np.concatenate([r["y"] for r in results], axis=0)


# revision 14
# speedup vs baseline: 2.8250x; 1.0245x over previous
"""Binarized dense layer for Trainium2 (8 NeuronCores, data-parallel).

Computes y = sign(x) @ sign(w) + b with sign(v) = -1 if v < 0 else +1,
matching jnp.where(v < 0, -1, 1) (including v == +0.0 -> +1).

Full shapes: x [8192, 2048] f32, w [2048, 2048] f32, b [2048] f32
-> y [8192, 2048] f32. Rows of x are sharded across 8 cores; w, b are
replicated.

Design notes (trace-driven):
  - The kernel is jointly input-DMA-bound (24 MB f32 in per core at
    ~410 GB/s sustained) and PE-bound (256 fp8 DoubleRow matmuls at
    ~216 ns pitch + 128 f32 transposes at ~109 ns ~= 69 us), so the
    schedule keeps both saturated from t~0: x tiles and w quarter-chunks
    interleave on the sync HWDGE queue in exactly PE consumption order.
  - PE FIFO order = emission order: transposes of m-tile i+2 are
    interleaved between the first quarter's matmul groups, so the PE
    streams densely (no HAM re-throttle) from first x arrival to the
    last matmul.
  - Sign activations (w chunks) and PSUM sign-evacuations (x) share the
    scalar queue, emitted in data-arrival order; y outputs ride GpSimd
    SWDGE so they never contend with input loads for queue slots.
  - An AllGather-based tensor-parallel weight fetch was measured and
    rejected: each NRT collective costs ~20 us and they serialize
    (~140 us for 4), dwarfing the 14 MB of HBM traffic it saves.

Matmul: fp8e4 DoubleRow (256-row contraction per instruction; +-1
products exact, f32 PSUM accumulation, |sums| <= 2048 exact). 8
accumulating matmuls per (m-tile, quarter) PSUM bank, 6 opsum banks +
2 transpose banks. VectorE adds the partition-broadcast f32 bias
(rounding matches the reference exactly).
"""
import numpy as np

import concourse.bass as bass
import concourse.mybir as mybir
import concourse.tile as tile
from concourse import bacc
from concourse.masks import make_identity

F32 = mybir.dt.float32
FP8 = mybir.dt.float8e4
P = 128
NQT = 512
Sign = mybir.ActivationFunctionType.Sign


def _build_kernel(M=1024, K=2048, N=2048, n_cores=8,
                  xstage_bufs=8, tpsum_bufs=2, opsum_bufs=6,
                  osb_bufs=4, out_eng='gpsimd'):
    KS = K // P            # 16 k-subtiles
    KP = KS // 2           # 8 DoubleRow matmuls per group
    MT = M // P            # 8 m-tiles
    NQ = N // NQT          # 4 column quarters
    WG = 4                 # k-subtiles per w stage chunk

    nc = bacc.Bacc("TRN2", target_bir_lowering=False, debug=False,
                   num_devices=n_cores)
    x = nc.dram_tensor("x", [M, K], F32, kind="ExternalInput").ap()
    w = nc.dram_tensor("w", [K, N], F32, kind="ExternalInput").ap()
    b = nc.dram_tensor("b", [N], F32, kind="ExternalInput").ap()
    y = nc.dram_tensor("y", [M, N], F32, kind="ExternalOutput").ap()
    w_r = w.rearrange("(a p) n -> p a n", p=P)

    with tile.TileContext(nc) as tc:
        with (
            tc.tile_pool(name="cst", bufs=1) as cst,
            tc.tile_pool(name="wstage", bufs=6) as wstage,
            tc.tile_pool(name="wq", bufs=2) as wqp,
            tc.tile_pool(name="xstage", bufs=xstage_bufs) as xstage,
            tc.tile_pool(name="xbt", bufs=1) as xbtp,
            tc.tile_pool(name="osb", bufs=osb_bufs) as osbp,
            tc.tile_pool(name="tpsum", bufs=tpsum_bufs, space="PSUM") as tpsum,
            tc.tile_pool(name="opsum", bufs=opsum_bufs, space="PSUM") as opsum,
        ):
            eps = cst.tile([P, 1], F32, tag="eps")
            nc.vector.memset(eps[:], 1e-30)
            ident = cst.tile([P, P], F32, tag="ident")
            make_identity(nc, ident[:])
            bias_q = [cst.tile([P, NQT], F32, tag=f"bias{q}", name=f"bias{q}")
                      for q in range(NQ)]

            xbt = [xbtp.tile([P, KS, P], FP8, tag=f"xbt{mi}",
                             name=f"xbt{mi}") for mi in range(MT)]
            wq = [wqp.tile([P, KS, NQT], FP8, tag=f"wq{q % 2}",
                           name=f"wq{q}") for q in range(NQ)]
            xss = [None] * MT

            def load_x(mi):
                xs = xstage.tile([P, K], F32, tag="xs", name=f"xs{mi}")
                nc.sync.dma_start(xs[:], x[mi * P:(mi + 1) * P, :])
                xss[mi] = xs

            def load_w_chunk(q, g):
                ws = wstage.tile([P, WG, NQT], F32, tag="ws",
                                 name=f"ws{q}_{g}")
                nc.sync.dma_start(
                    ws[:], w_r[:, g * WG:(g + 1) * WG,
                               q * NQT:(q + 1) * NQT])
                return ws

            def sign_w_chunk(q, g, ws):
                nc.scalar.activation(wq[q][:, g * WG:(g + 1) * WG, :],
                                     ws[:], Sign, bias=eps[:])

            # ---- input stream: interleaved in PE-consumption order ----
            for q in range(NQ):
                nc.sync.dma_start(
                    bias_q[q][:],
                    b[None, q * NQT:(q + 1) * NQT].to_broadcast([P, NQT]))
            load_x(0)
            load_x(1)
            wchunks = {}
            for g in range(4):
                wchunks[(0, g)] = load_w_chunk(0, g)
            load_x(2)
            load_x(3)
            for g in range(4):
                wchunks[(1, g)] = load_w_chunk(1, g)
            load_x(4)
            load_x(5)
            for g in range(4):
                wchunks[(2, g)] = load_w_chunk(2, g)
            load_x(6)
            load_x(7)
            for g in range(4):
                wchunks[(3, g)] = load_w_chunk(3, g)

            # ---- X prep: PE transpose (4 blocks/bank) + Sign evac ----
            def prep_x(mi):
                for g0 in range(0, KS, 4):
                    pt = tpsum.tile([P, 4 * P], F32, tag="tp",
                                    name=f"tp{mi}_{g0}")
                    for j in range(4):
                        kj = g0 + j
                        nc.tensor.transpose(pt[:, j * P:(j + 1) * P],
                                            xss[mi][:, kj * P:(kj + 1) * P],
                                            ident[:])
                    nc.scalar.activation(
                        xbt[mi][:, g0:g0 + 4, :],
                        pt[:].rearrange("p (a m) -> p a m", a=4),
                        Sign, bias=eps[:])

            def mm_group(q, mi, oeng):
                op = opsum.tile([P, NQT], F32, tag="op", name=f"op{mi}_{q}")
                for t in range(KP):
                    nc.tensor.matmul(
                        op[:],
                        lhsT=xbt[mi][:, 2 * t:2 * t + 2, :],
                        rhs=wq[q][:, 2 * t:2 * t + 2, :],
                        start=(t == 0), stop=(t == KP - 1),
                        perf_mode=mybir.MatmulPerfMode.DoubleRow)
                ob = osbp.tile([P, NQT], F32, tag="ob", name=f"ob{mi}_{q}")
                nc.vector.tensor_add(ob[:], op[:], bias_q[q][:])
                getattr(nc, oeng).dma_start(
                    y[mi * P:(mi + 1) * P, q * NQT:(q + 1) * NQT], ob[:])

            # Scalar FIFO: wsign0, then x evacs (critical path), then
            # wsign1..3 (needed only at each q phase start), then half the
            # y outputs. PE FIFO: preps interleaved with q0 groups.
            for g in range(4):
                sign_w_chunk(0, g, wchunks[(0, g)])
            prep_x(0)
            prep_x(1)
            for mi in range(MT):
                if mi + 2 < MT:
                    prep_x(mi + 2)
                mm_group(0, mi, 'gpsimd')
            for q in range(1, NQ):
                for g in range(4):
                    sign_w_chunk(q, g, wchunks[(q, g)])
            for q in range(1, NQ):
                for mi in range(MT):
                    mm_group(q, mi, 'gpsimd' if mi % 2 == 0 else 'scalar')
    nc.compile()
    return nc


N_CORES = 8
M_FULL, K_DIM, N_DIM = 8192, 2048, 2048
M_LOC = M_FULL // N_CORES
K_LOC = K_DIM // N_CORES
TP_W = False
_nc_cache = {}


def _get_nc():
    if "nc" not in _nc_cache:
        _nc_cache["nc"] = _build_kernel(M=M_LOC, K=K_DIM, N=N_DIM,
                                        n_cores=N_CORES)
    return _nc_cache["nc"]


def _in_maps(inputs, kernel, bias):
    return [{"x": inputs[c * M_LOC:(c + 1) * M_LOC, :],
             "w": kernel, "b": bias} for c in range(N_CORES)]


def kernel(inputs: np.ndarray, kernel: np.ndarray, bias: np.ndarray) -> np.ndarray:
    assert inputs.shape == (M_FULL, K_DIM) and inputs.dtype == np.float32
    assert kernel.shape == (K_DIM, N_DIM) and kernel.dtype == np.float32
    assert bias.shape == (N_DIM,) and bias.dtype == np.float32
    nc = _get_nc()
    in_maps = _in_maps(inputs, kernel, bias)
    try:
        from concourse.bass_utils import run_bass_kernel_spmd
        results = run_bass_kernel_spmd(
            nc, in_maps, core_ids=list(range(N_CORES))).results
    except Exception:
        from concourse import bass2jax
        bass2jax.install_neuronx_cc_hook()
        results = bass2jax.run_bass_via_pjrt(nc, in_maps, n_cores=N_CORES)
    return np.concatenate([r["y"] for r in results], axis=0)


# revision 18
# speedup vs baseline: 2.9389x; 1.0403x over previous
"""Binarized dense layer for Trainium2 (8 NeuronCores, data-parallel).

Computes y = sign(x) @ sign(w) + b with sign(v) = -1 if v < 0 else +1,
matching jnp.where(v < 0, -1, 1) (including v == +0.0 -> +1).

Full shapes: x [8192, 2048] f32, w [2048, 2048] f32, b [2048] f32
-> y [8192, 2048] f32. Rows of x are sharded across 8 cores; w, b are
replicated.

Design notes (trace-driven):
  - The kernel is jointly input-DMA-bound (24 MB f32 in per core at
    ~410 GB/s sustained) and PE-bound (256 fp8 DoubleRow matmuls at
    ~216 ns pitch + 128 f32 transposes at ~109 ns ~= 69 us), so the
    schedule keeps both saturated from t~0: x tiles and w quarter-chunks
    interleave on the sync HWDGE queue in exactly PE consumption order.
  - PE FIFO order = emission order: transposes of m-tile i+2 are
    interleaved between the first quarter's matmul groups, so the PE
    streams densely (no HAM re-throttle) from first x arrival to the
    last matmul.
  - Sign activations (w chunks) and PSUM sign-evacuations (x) share the
    scalar queue, emitted in data-arrival order; y outputs ride GpSimd
    SWDGE so they never contend with input loads for queue slots.
  - An AllGather-based tensor-parallel weight fetch was measured and
    rejected: each NRT collective costs ~20 us and they serialize
    (~140 us for 4), dwarfing the 14 MB of HBM traffic it saves.

Matmul: fp8e4 DoubleRow (256-row contraction per instruction; +-1
products exact, f32 PSUM accumulation, |sums| <= 2048 exact). 8
accumulating matmuls per (m-tile, quarter) PSUM bank, 6 opsum banks +
2 transpose banks. VectorE adds the partition-broadcast f32 bias
(rounding matches the reference exactly).
"""
import numpy as np

import concourse.bass as bass
import concourse.mybir as mybir
import concourse.tile as tile
from concourse import bacc
from concourse.masks import make_identity

F32 = mybir.dt.float32
FP8 = mybir.dt.float8e4
P = 128
NQT = 512
Sign = mybir.ActivationFunctionType.Sign


def _build_kernel(M=1024, K=2048, N=2048, n_cores=8,
                  xstage_bufs=7, tpsum_bufs=2, opsum_bufs=6,
                  osb_bufs=1, out_eng='gpsimd'):
    KS = K // P            # 16 k-subtiles
    KP = KS // 2           # 8 DoubleRow matmuls per group
    MT = M // P            # 8 m-tiles
    NQ = N // NQT          # 4 column quarters
    WG = 4                 # k-subtiles per w stage chunk

    nc = bacc.Bacc("TRN2", target_bir_lowering=False, debug=False,
                   num_devices=n_cores)
    x = nc.dram_tensor("x", [M, K], F32, kind="ExternalInput").ap()
    w = nc.dram_tensor("w", [K, N], F32, kind="ExternalInput").ap()
    b = nc.dram_tensor("b", [N], F32, kind="ExternalInput").ap()
    y = nc.dram_tensor("y", [M, N], F32, kind="ExternalOutput").ap()
    w_r = w.rearrange("(a p) n -> p a n", p=P)

    with tile.TileContext(nc) as tc:
        with (
            tc.tile_pool(name="cst", bufs=1) as cst,
            tc.tile_pool(name="wstage", bufs=5) as wstage,
            tc.tile_pool(name="wq", bufs=2) as wqp,
            tc.tile_pool(name="xstage", bufs=xstage_bufs) as xstage,
            tc.tile_pool(name="xbt", bufs=1) as xbtp,
            tc.tile_pool(name="osb", bufs=osb_bufs) as osbp,
            tc.tile_pool(name="tpsum", bufs=tpsum_bufs, space="PSUM") as tpsum,
            tc.tile_pool(name="opsum", bufs=opsum_bufs, space="PSUM") as opsum,
        ):
            eps = cst.tile([P, 1], F32, tag="eps")
            nc.vector.memset(eps[:], 1e-30)
            ident = cst.tile([P, P], F32, tag="ident")
            make_identity(nc, ident[:])
            bias_q = [cst.tile([P, NQT], F32, tag=f"bias{q}", name=f"bias{q}")
                      for q in range(NQ)]

            xbt = [xbtp.tile([P, KS, P], FP8, tag=f"xbt{mi}",
                             name=f"xbt{mi}") for mi in range(MT)]
            wq = [wqp.tile([P, KS, NQT], FP8, tag=f"wq{q % 2}",
                           name=f"wq{q}") for q in range(NQ)]
            xss = [None] * MT

            def load_x(mi):
                xs = xstage.tile([P, K], F32, tag="xs", name=f"xs{mi}")
                nc.sync.dma_start(xs[:], x[mi * P:(mi + 1) * P, :])
                xss[mi] = xs

            def load_w_chunk(q, g):
                ws = wstage.tile([P, WG, NQT], F32, tag="ws",
                                 name=f"ws{q}_{g}")
                nc.sync.dma_start(
                    ws[:], w_r[:, g * WG:(g + 1) * WG,
                               q * NQT:(q + 1) * NQT])
                return ws

            def sign_w_chunk(q, g, ws):
                nc.scalar.activation(wq[q][:, g * WG:(g + 1) * WG, :],
                                     ws[:], Sign, bias=eps[:])

            # ---- input stream: interleaved in PE-consumption order ----
            for q in range(NQ):
                nc.sync.dma_start(
                    bias_q[q][:],
                    b[None, q * NQT:(q + 1) * NQT].to_broadcast([P, NQT]))
            load_x(0)
            wchunks = {}
            wchunks[(0, 0)] = load_w_chunk(0, 0)
            load_x(1)
            for g in range(1, 4):
                wchunks[(0, g)] = load_w_chunk(0, g)
            load_x(2)
            load_x(3)
            for g in range(4):
                wchunks[(1, g)] = load_w_chunk(1, g)
            load_x(4)
            load_x(5)
            for g in range(4):
                wchunks[(2, g)] = load_w_chunk(2, g)
            load_x(6)
            load_x(7)
            for g in range(4):
                wchunks[(3, g)] = load_w_chunk(3, g)

            # ---- X prep: PE transpose (4 blocks/bank) + Sign evac ----
            def prep_x(mi):
                for g0 in range(0, KS, 4):
                    pt = tpsum.tile([P, 4 * P], F32, tag="tp",
                                    name=f"tp{mi}_{g0}")
                    for j in range(4):
                        kj = g0 + j
                        nc.tensor.transpose(pt[:, j * P:(j + 1) * P],
                                            xss[mi][:, kj * P:(kj + 1) * P],
                                            ident[:])
                    nc.scalar.activation(
                        xbt[mi][:, g0:g0 + 4, :],
                        pt[:].rearrange("p (a m) -> p a m", a=4),
                        Sign, bias=eps[:])

            # Paired output tiles: quarters (0,1) and (2,3) of an m-tile
            # accumulate into one [P, 2*NQT] SBUF tile, written to y with a
            # single DMA (4 KB runs instead of 2 KB -> much better write
            # descriptor efficiency).
            obs = {}

            def mm_group(q, mi, oeng):
                op = opsum.tile([P, NQT], F32, tag="op", name=f"op{mi}_{q}")
                for t in range(KP):
                    nc.tensor.matmul(
                        op[:],
                        lhsT=xbt[mi][:, 2 * t:2 * t + 2, :],
                        rhs=wq[q][:, 2 * t:2 * t + 2, :],
                        start=(t == 0), stop=(t == KP - 1),
                        perf_mode=mybir.MatmulPerfMode.DoubleRow)
                h = q // 2
                if q % 2 == 0:
                    obs[(h, mi)] = osbp.tile([P, 2 * NQT], F32,
                                             tag=f"ob{mi}",
                                             name=f"ob{mi}_{h}")
                ob = obs[(h, mi)]
                lo = (q % 2) * NQT
                nc.vector.tensor_add(ob[:, lo:lo + NQT], op[:], bias_q[q][:])
                if q % 2 == 1:
                    getattr(nc, oeng).dma_start(
                        y[mi * P:(mi + 1) * P,
                          h * 2 * NQT:(h + 1) * 2 * NQT], ob[:])

            # Scalar FIFO: wsign0, then x evacs (critical path), then
            # wsign1..3 (needed only at each q phase start), then half the
            # y outputs. PE FIFO: preps interleaved with q0 groups.
            for g in range(4):
                sign_w_chunk(0, g, wchunks[(0, g)])
            prep_x(0)
            prep_x(1)
            for mi in range(MT):
                if mi + 2 < MT:
                    prep_x(mi + 2)
                mm_group(0, mi, 'gpsimd')
            for q in range(1, NQ):
                for g in range(4):
                    sign_w_chunk(q, g, wchunks[(q, g)])
            for q in range(1, NQ):
                for mi in range(MT):
                    mm_group(q, mi, 'gpsimd' if mi % 2 == 0 else 'scalar')
    nc.compile()
    return nc


N_CORES = 8
M_FULL, K_DIM, N_DIM = 8192, 2048, 2048
M_LOC = M_FULL // N_CORES
K_LOC = K_DIM // N_CORES
TP_W = False
_nc_cache = {}


def _get_nc():
    if "nc" not in _nc_cache:
        _nc_cache["nc"] = _build_kernel(M=M_LOC, K=K_DIM, N=N_DIM,
                                        n_cores=N_CORES)
    return _nc_cache["nc"]


def _in_maps(inputs, kernel, bias):
    return [{"x": inputs[c * M_LOC:(c + 1) * M_LOC, :],
             "w": kernel, "b": bias} for c in range(N_CORES)]


def kernel(inputs: np.ndarray, kernel: np.ndarray, bias: np.ndarray) -> np.ndarray:
    assert inputs.shape == (M_FULL, K_DIM) and inputs.dtype == np.float32
    assert kernel.shape == (K_DIM, N_DIM) and kernel.dtype == np.float32
    assert bias.shape == (N_DIM,) and bias.dtype == np.float32
    nc = _get_nc()
    in_maps = _in_maps(inputs, kernel, bias)
    try:
        from concourse.bass_utils import run_bass_kernel_spmd
        results = run_bass_kernel_spmd(
            nc, in_maps, core_ids=list(range(N_CORES))).results
    except Exception:
        from concourse import bass2jax
        bass2jax.install_neuronx_cc_hook()
        results = bass2jax.run_bass_via_pjrt(nc, in_maps, n_cores=N_CORES)
    return np.concatenate([r["y"] for r in results], axis=0)


# revision 20
# speedup vs baseline: 2.9748x; 1.0122x over previous
"""Binarized dense layer for Trainium2 (8 NeuronCores, data-parallel).

Computes y = sign(x) @ sign(w) + b with sign(v) = -1 if v < 0 else +1,
matching jnp.where(v < 0, -1, 1) (including v == +0.0 -> +1).

Full shapes: x [8192, 2048] f32, w [2048, 2048] f32, b [2048] f32
-> y [8192, 2048] f32. Rows of x are sharded across 8 cores; w, b are
replicated.

Design notes (trace-driven):
  - The kernel is jointly input-DMA-bound (24 MB f32 in per core at
    ~410 GB/s sustained) and PE-bound (256 fp8 DoubleRow matmuls at
    ~216 ns pitch + 128 f32 transposes at ~109 ns ~= 69 us), so the
    schedule keeps both saturated from t~0: x tiles and w quarter-chunks
    interleave on the sync HWDGE queue in exactly PE consumption order.
  - PE FIFO order = emission order: transposes of m-tile i+2 are
    interleaved between the first quarter's matmul groups, so the PE
    streams densely (no HAM re-throttle) from first x arrival to the
    last matmul.
  - Sign activations (w chunks) and PSUM sign-evacuations (x) share the
    scalar queue, emitted in data-arrival order; y outputs ride GpSimd
    SWDGE so they never contend with input loads for queue slots.
  - An AllGather-based tensor-parallel weight fetch was measured and
    rejected: each NRT collective costs ~20 us and they serialize
    (~140 us for 4), dwarfing the 14 MB of HBM traffic it saves.

Matmul: fp8e4 DoubleRow (256-row contraction per instruction; +-1
products exact, f32 PSUM accumulation, |sums| <= 2048 exact). 8
accumulating matmuls per (m-tile, quarter) PSUM bank, 6 opsum banks +
2 transpose banks. VectorE adds the partition-broadcast f32 bias
(rounding matches the reference exactly).
"""
import numpy as np

import concourse.bass as bass
import concourse.mybir as mybir
import concourse.tile as tile
from concourse import bacc
from concourse.masks import make_identity

F32 = mybir.dt.float32
FP8 = mybir.dt.float8e4
P = 128
NQT = 512
Sign = mybir.ActivationFunctionType.Sign


def _build_kernel(M=1024, K=2048, N=2048, n_cores=8,
                  xstage_bufs=7, tpsum_bufs=2, opsum_bufs=6,
                  osb_bufs=1, out_eng='gpsimd'):
    KS = K // P            # 16 k-subtiles
    KP = KS // 2           # 8 DoubleRow matmuls per group
    MT = M // P            # 8 m-tiles
    NQ = N // NQT          # 4 column quarters
    WG = 4                 # k-subtiles per w stage chunk

    nc = bacc.Bacc("TRN2", target_bir_lowering=False, debug=False,
                   num_devices=n_cores)
    x = nc.dram_tensor("x", [M, K], F32, kind="ExternalInput").ap()
    w = nc.dram_tensor("w", [K, N], F32, kind="ExternalInput").ap()
    b = nc.dram_tensor("b", [N], F32, kind="ExternalInput").ap()
    y = nc.dram_tensor("y", [M, N], F32, kind="ExternalOutput").ap()
    w_r = w.rearrange("(a p) n -> p a n", p=P)

    with tile.TileContext(nc) as tc:
        with (
            tc.tile_pool(name="cst", bufs=1) as cst,
            tc.tile_pool(name="wstage", bufs=5) as wstage,
            tc.tile_pool(name="wq", bufs=2) as wqp,
            tc.tile_pool(name="xstage", bufs=xstage_bufs) as xstage,
            tc.tile_pool(name="xbt", bufs=1) as xbtp,
            tc.tile_pool(name="osb", bufs=osb_bufs) as osbp,
            tc.tile_pool(name="tpsum", bufs=tpsum_bufs, space="PSUM") as tpsum,
            tc.tile_pool(name="opsum", bufs=opsum_bufs, space="PSUM") as opsum,
        ):
            eps = cst.tile([P, 1], F32, tag="eps")
            nc.vector.memset(eps[:], 1e-30)
            ident = cst.tile([P, P], F32, tag="ident")
            make_identity(nc, ident[:])
            bias_q = [cst.tile([P, NQT], F32, tag=f"bias{q}", name=f"bias{q}")
                      for q in range(NQ)]

            xbt = [xbtp.tile([P, KS, P], FP8, tag=f"xbt{mi}",
                             name=f"xbt{mi}") for mi in range(MT)]
            wq = [wqp.tile([P, KS, NQT], FP8, tag=f"wq{q % 2}",
                           name=f"wq{q}") for q in range(NQ)]
            xss = [None] * MT

            def load_x(mi):
                xs = xstage.tile([P, K], F32, tag="xs", name=f"xs{mi}")
                nc.sync.dma_start(xs[:], x[mi * P:(mi + 1) * P, :])
                xss[mi] = xs

            def load_w_chunk(q, g):
                ws = wstage.tile([P, WG, NQT], F32, tag="ws",
                                 name=f"ws{q}_{g}")
                nc.sync.dma_start(
                    ws[:], w_r[:, g * WG:(g + 1) * WG,
                               q * NQT:(q + 1) * NQT])
                return ws

            def sign_w_chunk(q, g, ws):
                nc.scalar.activation(wq[q][:, g * WG:(g + 1) * WG, :],
                                     ws[:], Sign, bias=eps[:])

            # ---- input stream: interleaved in PE-consumption order ----
            # x tile i alternates with w chunk so that prep(i) and the
            # matmul quarter phases never wait on the sync queue.
            for q in range(NQ):
                nc.sync.dma_start(
                    bias_q[q][:],
                    b[None, q * NQT:(q + 1) * NQT].to_broadcast([P, NQT]))
            wchunks = {}
            load_x(0)
            wchunks[(0, 0)] = load_w_chunk(0, 0)
            load_x(1)
            wchunks[(0, 1)] = load_w_chunk(0, 1)
            load_x(2)
            wchunks[(0, 2)] = load_w_chunk(0, 2)
            load_x(3)
            wchunks[(0, 3)] = load_w_chunk(0, 3)
            load_x(4)
            wchunks[(1, 0)] = load_w_chunk(1, 0)
            load_x(5)
            wchunks[(1, 1)] = load_w_chunk(1, 1)
            load_x(6)
            wchunks[(1, 2)] = load_w_chunk(1, 2)
            load_x(7)
            wchunks[(1, 3)] = load_w_chunk(1, 3)
            for q in range(2, NQ):
                for g in range(4):
                    wchunks[(q, g)] = load_w_chunk(q, g)

            # ---- X prep: PE transpose (4 blocks/bank) + Sign evac ----
            def prep_x(mi):
                for g0 in range(0, KS, 4):
                    pt = tpsum.tile([P, 4 * P], F32, tag="tp",
                                    name=f"tp{mi}_{g0}")
                    for j in range(4):
                        kj = g0 + j
                        nc.tensor.transpose(pt[:, j * P:(j + 1) * P],
                                            xss[mi][:, kj * P:(kj + 1) * P],
                                            ident[:])
                    nc.scalar.activation(
                        xbt[mi][:, g0:g0 + 4, :],
                        pt[:].rearrange("p (a m) -> p a m", a=4),
                        Sign, bias=eps[:])

            # Paired output tiles: quarters (0,1) and (2,3) of an m-tile
            # accumulate into one [P, 2*NQT] SBUF tile, written to y with a
            # single DMA (4 KB runs instead of 2 KB -> much better write
            # descriptor efficiency).
            obs = {}

            def mm_group(q, mi, oeng):
                op = opsum.tile([P, NQT], F32, tag="op", name=f"op{mi}_{q}")
                for t in range(KP):
                    nc.tensor.matmul(
                        op[:],
                        lhsT=xbt[mi][:, 2 * t:2 * t + 2, :],
                        rhs=wq[q][:, 2 * t:2 * t + 2, :],
                        start=(t == 0), stop=(t == KP - 1),
                        perf_mode=mybir.MatmulPerfMode.DoubleRow)
                h = q // 2
                if q % 2 == 0:
                    obs[(h, mi)] = osbp.tile([P, 2 * NQT], F32,
                                             tag=f"ob{mi}",
                                             name=f"ob{mi}_{h}")
                ob = obs[(h, mi)]
                lo = (q % 2) * NQT
                nc.vector.tensor_add(ob[:, lo:lo + NQT], op[:], bias_q[q][:])
                if q % 2 == 1:
                    getattr(nc, oeng).dma_start(
                        y[mi * P:(mi + 1) * P,
                          h * 2 * NQT:(h + 1) * 2 * NQT], ob[:])

            # Scalar FIFO: wsign0, then x evacs (critical path), then
            # wsign1..3 (needed only at each q phase start), then half the
            # y outputs. PE FIFO: preps interleaved with q0 groups.
            for g in range(4):
                sign_w_chunk(0, g, wchunks[(0, g)])
            prep_x(0)
            prep_x(1)
            for mi in range(MT):
                if mi + 2 < MT:
                    prep_x(mi + 2)
                mm_group(0, mi, 'gpsimd')
            for q in range(1, NQ):
                for g in range(4):
                    sign_w_chunk(q, g, wchunks[(q, g)])
                for mi in range(MT):
                    mm_group(q, mi, 'gpsimd' if mi % 2 == 0 else 'scalar')
    nc.compile()
    return nc


N_CORES = 8
M_FULL, K_DIM, N_DIM = 8192, 2048, 2048
M_LOC = M_FULL // N_CORES
K_LOC = K_DIM // N_CORES
TP_W = False
_nc_cache = {}


def _get_nc():
    if "nc" not in _nc_cache:
        _nc_cache["nc"] = _build_kernel(M=M_LOC, K=K_DIM, N=N_DIM,
                                        n_cores=N_CORES)
    return _nc_cache["nc"]


def _in_maps(inputs, kernel, bias):
    return [{"x": inputs[c * M_LOC:(c + 1) * M_LOC, :],
             "w": kernel, "b": bias} for c in range(N_CORES)]


def kernel(inputs: np.ndarray, kernel: np.ndarray, bias: np.ndarray) -> np.ndarray:
    assert inputs.shape == (M_FULL, K_DIM) and inputs.dtype == np.float32
    assert kernel.shape == (K_DIM, N_DIM) and kernel.dtype == np.float32
    assert bias.shape == (N_DIM,) and bias.dtype == np.float32
    nc = _get_nc()
    in_maps = _in_maps(inputs, kernel, bias)
    try:
        from concourse.bass_utils import run_bass_kernel_spmd
        results = run_bass_kernel_spmd(
            nc, in_maps, core_ids=list(range(N_CORES))).results
    except Exception:
        from concourse import bass2jax
        bass2jax.install_neuronx_cc_hook()
        results = bass2jax.run_bass_via_pjrt(nc, in_maps, n_cores=N_CORES)
    return np.concatenate([r["y"] for r in results], axis=0)


# revision 21
# speedup vs baseline: 3.0951x; 1.0404x over previous
"""Binarized dense layer for Trainium2 (8 NeuronCores, data-parallel).

Computes y = sign(x) @ sign(w) + b  with sign(v) = -1 if v < 0 else +1,
matching jnp.where(v < 0, -1, 1) bit-exactly (including v == +0.0 -> +1).

Full shapes: x [8192, 2048] f32, w [2048, 2048] f32, b [2048] f32
-> y [8192, 2048] f32. Rows of x are sharded across 8 cores; w, b are
replicated. Per-core kernel design:

  X path: DMA f32 row-chunks -> PE transpose-mode (128x128 f32 blocks,
      4 per PSUM bank) -> ScalarE Sign (+1e-30 bias so sign(0)=+1)
      evacuates to fp8e4 +-1 tiles in [k-partition, k-subtile, m] layout.
  W path: streamed by 512-column quarters (so each PSUM accumulation
      group's weights arrive k-complete early): DMA f32 -> ScalarE Sign
      -> fp8e4 quad tiles. Each quarter's bias slice is DMA-broadcast
      alongside it.
  Matmul: fp8 DoubleRow (256-row contraction per instruction; +-1
      products exact, fp32 PSUM accumulation, |sums| <= 2048 exact).
      8 accumulating matmuls per (m-tile, quarter) into one PSUM bank.
      The (quarter, m-tile) schedule interleaves q0/q1 m-blocks around
      the X stream to avoid PE FIFO head-of-line stalls on late X tiles.
  Epilogue: VectorE tensor_tensor adds the partition-replicated f32 bias
      (rounding matches the reference exactly); output DMAs issue from
      GPSIMD (SWDGE) so they never block input loads on the Sync queue.
"""
import numpy as np


import concourse.bass as bass
import concourse.mybir as mybir
import concourse.tile as tile
from concourse import bacc
from concourse.masks import make_identity

F32 = mybir.dt.float32
FP8 = mybir.dt.float8e4
P = 128
NQT = 512
Sign = mybir.ActivationFunctionType.Sign


def _build_kernel(M=1024, K=2048, N=2048, n_cores=8, xstage_bufs=5, wstage_bufs=6,
                 wq_bufs=3, tpsum_bufs=2, opsum_bufs=4, osb_bufs=4, tg=4,
                 out_eng='gpsimd', split=5, bias_eng='gpsimd', bias_early=True,
                 phase_barrier=False):
    KS = K // P
    KP = KS // 2
    MT = M // P
    NQ = N // NQT
    WG = 4
    NQUAD = KS // WG
    XQ = KS // tg            # xbt quads per m-tile
    nc = bacc.Bacc("TRN2", target_bir_lowering=False, debug=False, num_devices=n_cores)
    x = nc.dram_tensor("x", [M, K], F32, kind="ExternalInput").ap()
    w = nc.dram_tensor("w", [K, N], F32, kind="ExternalInput").ap()
    b = nc.dram_tensor("b", [N], F32, kind="ExternalInput").ap()
    y = nc.dram_tensor("y", [M, N], F32, kind="ExternalOutput").ap()
    w_r = w.rearrange("(a p) n -> p a n", p=P)

    with tile.TileContext(nc) as tc:
        with (
            tc.tile_pool(name="cst", bufs=1) as cst,
            tc.tile_pool(name="xstage", bufs=xstage_bufs) as xstage,
            tc.tile_pool(name="xbt", bufs=1) as xbtp,
            tc.tile_pool(name="wstage", bufs=wstage_bufs) as wstage,
            tc.tile_pool(name="wq", bufs=wq_bufs) as wqp,
            tc.tile_pool(name="osb", bufs=osb_bufs) as osbp,
            tc.tile_pool(name="tpsum", bufs=tpsum_bufs, space="PSUM") as tpsum,
            tc.tile_pool(name="opsum", bufs=opsum_bufs, space="PSUM") as opsum,
        ):
            eps = cst.tile([P, 1], F32, tag="eps")
            nc.vector.memset(eps[:], 1e-30)
            ident = cst.tile([P, P], F32, tag="ident")
            make_identity(nc, ident[:])
            bias_q = [cst.tile([P, NQT], F32, tag=f"bias{q}", name=f"bias{q}")
                      for q in range(NQ)]

            xbt = [[xbtp.tile([P, tg, P], FP8, tag=f"xbt{mi}_{g}",
                              name=f"xbt{mi}_{g}") for g in range(XQ)]
                   for mi in range(MT)]

            def load_x(mi):
                xs = xstage.tile([P, K], F32, tag="xs", name=f"xs{mi}")
                nc.sync.dma_start(xs[:], x[mi * P:(mi + 1) * P, :])
                return xs

            last_prep = [None]

            def prep_x(mi, xs):
                for g in range(XQ):
                    pt = tpsum.tile([P, tg * P], F32, tag="tp", name=f"tp{mi}_{g}")
                    for j in range(tg):
                        kj = g * tg + j
                        nc.tensor.transpose(pt[:, j * P:(j + 1) * P],
                                            xs[:, kj * P:(kj + 1) * P], ident[:])
                    last_prep[0] = nc.scalar.activation(
                        xbt[mi][g][:],
                        pt[:].rearrange("p (a m) -> p a m", a=tg),
                        Sign, bias=eps[:])

            def lhs_pair(mi, t):
                g, h = (2 * t) // tg, (2 * t) % tg
                return xbt[mi][g][:, h:h + 2, :]

            def load_wq(q):
                nc.sync.dma_start(
                    bias_q[q][:],
                    b[None, q * NQT:(q + 1) * NQT].to_broadcast([P, NQT]))
                quads = []
                for g in range(NQUAD):
                    ws = wstage.tile([P, WG, NQT], F32, tag="ws", name=f"ws{q}_{g}")
                    nc.sync.dma_start(
                        ws[:], w_r[:, g * WG:(g + 1) * WG,
                                   q * NQT:(q + 1) * NQT])
                    wqt = wqp.tile([P, WG, NQT], FP8, tag=f"wqt{g}",
                                   name=f"wq{q}_{g}")
                    nc.scalar.activation(wqt[:], ws[:], Sign, bias=eps[:])
                    quads.append(wqt)
                return quads

            def rhs_pair(quads, t):
                g, h = t // (WG // 2), t % (WG // 2)
                return quads[g][:, 2 * h:2 * h + 2, :]

            xs0 = load_x(0)
            wq_tiles = {0: load_wq(0)}
            prep_x(0, xs0)
            for mi in range(1, MT):
                xs = load_x(mi)
                if mi == min(2, MT - 1) and NQ > 1:
                    wq_tiles[1] = load_wq(1)
                prep_x(mi, xs)
            if 1 not in wq_tiles and NQ > 1:
                wq_tiles[1] = load_wq(1)

            # schedule: interleave q0/q1 around the X stream, then q2, q3
            if NQ >= 2 and MT > split:
                sched = [(0, mi) for mi in range(split)]
                sched += [(1, mi) for mi in range(split)]
                sched += [(0, mi) for mi in range(split, MT)]
                sched += [(1, mi) for mi in range(split, MT)]
                for q in range(2, NQ):
                    sched += [(q, mi) for mi in range(MT)]
            else:
                sched = [(q, mi) for q in range(NQ) for mi in range(MT)]
            prefetch_at = {}
            if NQ > 2:
                # emit load_wq(q+2) when q first appears in sched
                seen = set()
                for idx, (q, mi) in enumerate(sched):
                    if q not in seen:
                        seen.add(q)
                        if q + 2 < NQ:
                            prefetch_at[idx] = q + 2

            for idx, (q, mi) in enumerate(sched):
                if idx in prefetch_at:
                    wq_tiles[prefetch_at[idx]] = load_wq(prefetch_at[idx])
                quads = wq_tiles[q]
                op = opsum.tile([P, NQT], F32, tag="op", name=f"op{mi}_{q}")
                for t in range(KP):
                    h = nc.tensor.matmul(
                        op[:],
                        lhsT=lhs_pair(mi, t),
                        rhs=rhs_pair(quads, t),
                        start=(t == 0), stop=(t == KP - 1),
                        perf_mode=mybir.MatmulPerfMode.DoubleRow)
                    if phase_barrier and idx == 0 and t == 0 and last_prep[0] is not None:
                        from concourse.tile import add_dep_helper
                        add_dep_helper(h.ins, last_prep[0].ins, sync=True,
                                       reason="phase barrier: MMs after X prep")
                ob = osbp.tile([P, NQT], F32, tag="ob", name=f"ob{mi}_{q}")
                nc.vector.tensor_add(ob[:], op[:], bias_q[q][:])
                getattr(nc, out_eng).dma_start(
                    y[mi * P:(mi + 1) * P, q * NQT:(q + 1) * NQT], ob[:])
    nc.compile()
    return nc


N_CORES = 8
M_FULL, K_DIM, N_DIM = 8192, 2048, 2048
M_LOC = M_FULL // N_CORES
_nc_cache = {}


def _get_nc():
    if "nc" not in _nc_cache:
        _nc_cache["nc"] = _build_kernel(M=M_LOC, K=K_DIM, N=N_DIM,
                                        n_cores=N_CORES, split=6)
    return _nc_cache["nc"]


def kernel(inputs: np.ndarray, kernel: np.ndarray, bias: np.ndarray) -> np.ndarray:
    assert inputs.shape == (M_FULL, K_DIM) and inputs.dtype == np.float32
    assert kernel.shape == (K_DIM, N_DIM) and kernel.dtype == np.float32
    assert bias.shape == (N_DIM,) and bias.dtype == np.float32
    nc = _get_nc()
    in_maps = [
        {"x": inputs[c * M_LOC:(c + 1) * M_LOC, :], "w": kernel, "b": bias}
        for c in range(N_CORES)
    ]
    try:
        from concourse.bass_utils import run_bass_kernel_spmd
        results = run_bass_kernel_spmd(
            nc, in_maps, core_ids=list(range(N_CORES))).results
    except Exception:
        from concourse import bass2jax
        bass2jax.install_neuronx_cc_hook()
        results = bass2jax.run_bass_via_pjrt(nc, in_maps, n_cores=N_CORES)
    return np.concatenate([r["y"] for r in results], axis=0)



# revision 24
# speedup vs baseline: 3.1239x; 1.0093x over previous
"""Binarized dense layer for Trainium2 (8 NeuronCores, data-parallel).

Computes y = sign(x) @ sign(w) + b with sign(v) = -1 if v < 0 else +1,
matching jnp.where(v < 0, -1, 1) (including v == +0.0 -> +1).

Full shapes: x [8192, 2048] f32, w [2048, 2048] f32, b [2048] f32
-> y [8192, 2048] f32. Rows of x are sharded across 8 cores; w, b are
replicated.

Design notes (trace-driven):
  - The kernel is jointly input-DMA-bound (24 MB f32 in per core at
    ~410 GB/s sustained) and PE-bound (256 fp8 DoubleRow matmuls at
    ~216 ns pitch + 128 f32 transposes at ~109 ns ~= 69 us), so the
    schedule keeps both saturated from t~0: x tiles and w quarter-chunks
    interleave on the sync HWDGE queue in exactly PE consumption order.
  - PE FIFO order = emission order: transposes of m-tile i+2 are
    interleaved between the first quarter's matmul groups, so the PE
    streams densely (no HAM re-throttle) from first x arrival to the
    last matmul.
  - Sign activations (w chunks) and PSUM sign-evacuations (x) share the
    scalar queue, emitted in data-arrival order; y outputs ride GpSimd
    SWDGE so they never contend with input loads for queue slots.
  - An AllGather-based tensor-parallel weight fetch was measured and
    rejected: each NRT collective costs ~20 us and they serialize
    (~140 us for 4), dwarfing the 14 MB of HBM traffic it saves.

Matmul: fp8e4 DoubleRow (256-row contraction per instruction; +-1
products exact, f32 PSUM accumulation, |sums| <= 2048 exact). 8
accumulating matmuls per (m-tile, quarter) PSUM bank, 6 opsum banks +
2 transpose banks. VectorE adds the partition-broadcast f32 bias
(rounding matches the reference exactly).
"""
import numpy as np

import concourse.bass as bass
import concourse.mybir as mybir
import concourse.tile as tile
from concourse import bacc
from concourse.masks import make_identity

F32 = mybir.dt.float32
FP8 = mybir.dt.float8e4
P = 128
NQT = 512
Sign = mybir.ActivationFunctionType.Sign


def _build_kernel(M=1024, K=2048, N=2048, n_cores=8,
                  xstage_bufs=7, tpsum_bufs=2, opsum_bufs=6,
                  osb_bufs=4, out_eng='gpsimd'):
    KS = K // P            # 16 k-subtiles
    KP = KS // 2           # 8 DoubleRow matmuls per group
    MT = M // P            # 8 m-tiles
    NQ = N // NQT          # 4 column quarters
    WG = 4                 # k-subtiles per w stage chunk

    nc = bacc.Bacc("TRN2", target_bir_lowering=False, debug=False,
                   num_devices=n_cores)
    x = nc.dram_tensor("x", [M, K], F32, kind="ExternalInput").ap()
    w = nc.dram_tensor("w", [K, N], F32, kind="ExternalInput").ap()
    b = nc.dram_tensor("b", [N], F32, kind="ExternalInput").ap()
    y = nc.dram_tensor("y", [M, N], F32, kind="ExternalOutput").ap()
    w_r = w.rearrange("(a p) n -> p a n", p=P)

    with tile.TileContext(nc) as tc:
        with (
            tc.tile_pool(name="cst", bufs=1) as cst,
            tc.tile_pool(name="wstage", bufs=5) as wstage,
            tc.tile_pool(name="wq", bufs=2) as wqp,
            tc.tile_pool(name="xstage", bufs=xstage_bufs) as xstage,
            tc.tile_pool(name="xbt", bufs=1) as xbtp,
            tc.tile_pool(name="osb", bufs=osb_bufs) as osbp,
            tc.tile_pool(name="tpsum", bufs=tpsum_bufs, space="PSUM") as tpsum,
            tc.tile_pool(name="opsum", bufs=opsum_bufs, space="PSUM") as opsum,
        ):
            eps = cst.tile([P, 1], F32, tag="eps")
            nc.vector.memset(eps[:], 1e-30)
            ident = cst.tile([P, P], F32, tag="ident")
            make_identity(nc, ident[:])
            bias_q = [cst.tile([P, NQT], F32, tag=f"bias{q}", name=f"bias{q}")
                      for q in range(NQ)]

            xbt = [xbtp.tile([P, KS, P], FP8, tag=f"xbt{mi}",
                             name=f"xbt{mi}") for mi in range(MT)]
            wq = [wqp.tile([P, KS, NQT], FP8, tag=f"wq{q % 2}",
                           name=f"wq{q}") for q in range(NQ)]
            xss = [None] * MT

            def load_x(mi):
                xs = xstage.tile([P, K], F32, tag="xs", name=f"xs{mi}")
                nc.sync.dma_start(xs[:], x[mi * P:(mi + 1) * P, :])
                xss[mi] = xs

            def load_w_chunk(q, g):
                ws = wstage.tile([P, WG, NQT], F32, tag="ws",
                                 name=f"ws{q}_{g}")
                nc.sync.dma_start(
                    ws[:], w_r[:, g * WG:(g + 1) * WG,
                               q * NQT:(q + 1) * NQT])
                return ws

            def sign_w_chunk(q, g, ws):
                nc.scalar.activation(wq[q][:, g * WG:(g + 1) * WG, :],
                                     ws[:], Sign, bias=eps[:])

            # ---- input stream: interleaved in PE-consumption order ----
            # x tile i alternates with w chunk so that prep(i) and the
            # matmul quarter phases never wait on the sync queue.
            for q in range(NQ):
                nc.sync.dma_start(
                    bias_q[q][:],
                    b[None, q * NQT:(q + 1) * NQT].to_broadcast([P, NQT]))
            wchunks = {}
            load_x(0)
            wchunks[(0, 0)] = load_w_chunk(0, 0)
            load_x(1)
            wchunks[(0, 1)] = load_w_chunk(0, 1)
            load_x(2)
            wchunks[(0, 2)] = load_w_chunk(0, 2)
            load_x(3)
            wchunks[(0, 3)] = load_w_chunk(0, 3)
            load_x(4)
            wchunks[(1, 0)] = load_w_chunk(1, 0)
            load_x(5)
            wchunks[(1, 1)] = load_w_chunk(1, 1)
            load_x(6)
            wchunks[(1, 2)] = load_w_chunk(1, 2)
            load_x(7)
            wchunks[(1, 3)] = load_w_chunk(1, 3)
            for q in range(2, NQ):
                for g in range(4):
                    wchunks[(q, g)] = load_w_chunk(q, g)

            # ---- X prep: PE transpose (4 blocks/bank) + Sign evac ----
            def prep_x(mi):
                for g0 in range(0, KS, 4):
                    pt = tpsum.tile([P, 4 * P], F32, tag="tp",
                                    name=f"tp{mi}_{g0}")
                    for j in range(4):
                        kj = g0 + j
                        nc.tensor.transpose(pt[:, j * P:(j + 1) * P],
                                            xss[mi][:, kj * P:(kj + 1) * P],
                                            ident[:])
                    nc.scalar.activation(
                        xbt[mi][:, g0:g0 + 4, :],
                        pt[:].rearrange("p (a m) -> p a m", a=4),
                        Sign, bias=eps[:])

            # Per-quarter outputs, fired immediately after each group's
            # bias add so the y stream drains *during* the matmul phases
            # and only the last group's 256 KB trails the final matmul.
            def mm_group(q, mi, oeng):
                op = opsum.tile([P, NQT], F32, tag="op", name=f"op{mi}_{q}")
                for t in range(KP):
                    nc.tensor.matmul(
                        op[:],
                        lhsT=xbt[mi][:, 2 * t:2 * t + 2, :],
                        rhs=wq[q][:, 2 * t:2 * t + 2, :],
                        start=(t == 0), stop=(t == KP - 1),
                        perf_mode=mybir.MatmulPerfMode.DoubleRow)
                ob = osbp.tile([P, NQT], F32, tag="ob", name=f"ob{mi}_{q}")
                nc.vector.tensor_add(ob[:], op[:], bias_q[q][:])
                getattr(nc, oeng).dma_start(
                    y[mi * P:(mi + 1) * P, q * NQT:(q + 1) * NQT], ob[:])

            # Scalar FIFO: wsign0, then x evacs (critical path), then
            # wsign1..3 (needed only at each q phase start), then half the
            # y outputs. PE FIFO: preps interleaved with q0 groups.
            for g in range(4):
                sign_w_chunk(0, g, wchunks[(0, g)])
            prep_x(0)
            prep_x(1)
            for mi in range(MT):
                if mi + 2 < MT:
                    prep_x(mi + 2)
                mm_group(0, mi, 'gpsimd')
            for q in range(1, NQ):
                for g in range(4):
                    sign_w_chunk(q, g, wchunks[(q, g)])
                for mi in range(MT):
                    mm_group(q, mi, 'gpsimd' if mi % 2 == 0 else 'scalar')
    nc.compile()
    return nc


N_CORES = 8
M_FULL, K_DIM, N_DIM = 8192, 2048, 2048
M_LOC = M_FULL // N_CORES
K_LOC = K_DIM // N_CORES
TP_W = False
_nc_cache = {}


def _get_nc():
    if "nc" not in _nc_cache:
        _nc_cache["nc"] = _build_kernel(M=M_LOC, K=K_DIM, N=N_DIM,
                                        n_cores=N_CORES)
    return _nc_cache["nc"]


def _in_maps(inputs, kernel, bias):
    return [{"x": inputs[c * M_LOC:(c + 1) * M_LOC, :],
             "w": kernel, "b": bias} for c in range(N_CORES)]


def kernel(inputs: np.ndarray, kernel: np.ndarray, bias: np.ndarray) -> np.ndarray:
    assert inputs.shape == (M_FULL, K_DIM) and inputs.dtype == np.float32
    assert kernel.shape == (K_DIM, N_DIM) and kernel.dtype == np.float32
    assert bias.shape == (N_DIM,) and bias.dtype == np.float32
    nc = _get_nc()
    in_maps = _in_maps(inputs, kernel, bias)
    try:
        from concourse.bass_utils import run_bass_kernel_spmd
        results = run_bass_kernel_spmd(
            nc, in_maps, core_ids=list(range(N_CORES))).results
    except Exception:
        from concourse import bass2jax
        bass2jax.install_neuronx_cc_hook()
        results = bass2jax.run_bass_via_pjrt(nc, in_maps, n_cores=N_CORES)
    return np.concatenate([r["y"] for r in results], axis=0)
